# revision 1
# baseline (speedup 1.0000x reference)
"""GaussianMLP sampling kernel for 8 trn2 NeuronCores (pure data parallel).

reference:
    h      = relu(x @ W_emb + b_emb)        x:[B,128] W_emb:[128,256]
    mean   = h @ W_mean + b_mean            W_mean:[256,128]
    logvar = h @ W_logvar + b_logvar        W_logvar:[256,128]
    z      = mean + exp(0.5*logvar) * eps
    returns (z, mean, logvar)

Sharding: x/eps split along batch across 8 cores; weights replicated.

Per-core dataflow (ROWS_PER_TILE=512 rows/iteration):
  - DMA x tile [128p, 4, 128] (natural layout, partition=row)
  - PE transpose 4x [128,128] -> xT [d_in=128p, 512 rows] in PSUM, copy to SBUF
  - hT = W_emb.T @ x.T via 2 matmuls (lhsT=W_emb chunk, rhs=xT) -> PSUM
  - ACT relu(hT + b_emb) PSUM->SBUF (per-partition bias)
  - mean/logvar: bias seeded by a K=1 ones-matmul over the whole PSUM bank,
    then per-128-row subtile: accumulate hT0.T@Wm0 + hT1.T@Wm1
  - epilogue: ACT copies logvar out + exp(0.5*logvar); DVE mean copy,
    se=std*eps, z=mean+se; DMA 3 outputs
"""

import os
import sys

sys.path.insert(0, "/opt/trn_rl_repo")

import numpy as np

from contextlib import ExitStack

from concourse import bacc, bass, masks, mybir, tile
from concourse.alu_op_type import AluOpType
from concourse.bass_utils import run_bass_kernel_spmd

N_CORES = 8
B = 524288
D_IN = 128
D_H = 256
D_OUT = 128
ROWS_PER_CORE = B // N_CORES  # 65536
ROWS_PER_TILE = 512
N_TILES = ROWS_PER_CORE // ROWS_PER_TILE  # 128
S = ROWS_PER_TILE // 128  # 4 subtiles of 128 rows

F32 = mybir.dt.float32
# dtype for the two matmul layers (float32 = exact-ish, bfloat16 = faster PE)
BF16 = mybir.dt.bfloat16
L1_DT = BF16
L2_DT = BF16


def build_bass(rows_per_core=ROWS_PER_CORE):
    nc = bacc.Bacc("TRN2", target_bir_lowering=False, debug=False)
    n_tiles = rows_per_core // ROWS_PER_TILE

    x_ext = nc.declare_dram_parameter("x", [rows_per_core, D_IN], F32, isOutput=False)
    eps_ext = nc.declare_dram_parameter(
        "eps", [rows_per_core, D_OUT], F32, isOutput=False
    )
    We_ext = nc.declare_dram_parameter("W_emb", [D_IN, D_H], F32, isOutput=False)
    be_ext = nc.declare_dram_parameter("b_emb", [D_H], F32, isOutput=False)
    Wm_ext = nc.declare_dram_parameter("W_mean", [D_H, D_OUT], F32, isOutput=False)
    bm_ext = nc.declare_dram_parameter("b_mean", [D_OUT], F32, isOutput=False)
    Wl_ext = nc.declare_dram_parameter("W_logvar", [D_H, D_OUT], F32, isOutput=False)
    bl_ext = nc.declare_dram_parameter("b_logvar", [D_OUT], F32, isOutput=False)
    z_ext = nc.declare_dram_parameter("z", [rows_per_core, D_OUT], F32, isOutput=True)
    mean_ext = nc.declare_dram_parameter(
        "mean", [rows_per_core, D_OUT], F32, isOutput=True
    )
    lv_ext = nc.declare_dram_parameter(
        "logvar", [rows_per_core, D_OUT], F32, isOutput=True
    )

    # tiled DRAM views: row = t*ROWS_PER_TILE + s*128 + p
    xv = x_ext.rearrange("(t s p) d -> t p s d", s=S, p=128)
    ev = eps_ext.rearrange("(t s p) d -> t p s d", s=S, p=128)
    zv = z_ext.rearrange("(t s p) d -> t p s d", s=S, p=128)
    mv = mean_ext.rearrange("(t s p) d -> t p s d", s=S, p=128)
    lvv = lv_ext.rearrange("(t s p) d -> t p s d", s=S, p=128)

    with tile.TileContext(nc) as tc, ExitStack() as ctx:
        const = ctx.enter_context(tc.tile_pool(name="const", bufs=1))
        xin = ctx.enter_context(tc.tile_pool(name="xin", bufs=3))
        epool = ctx.enter_context(tc.tile_pool(name="eps", bufs=3))
        xTp = ctx.enter_context(tc.tile_pool(name="xT", bufs=3))
        hTp = ctx.enter_context(tc.tile_pool(name="hTs", bufs=2))
        outs = ctx.enter_context(tc.tile_pool(name="outs", bufs=3))
        psA = ctx.enter_context(tc.tile_pool(name="psA", bufs=2, space="PSUM"))
        psB = ctx.enter_context(tc.tile_pool(name="psB", bufs=1, space="PSUM"))
        psC = ctx.enter_context(tc.tile_pool(name="psC", bufs=2, space="PSUM"))

        # --- constants / weights (loaded once) ---
        ident = const.tile([128, 128], F32)
        masks.make_identity(nc, ident[:])

        We_sb = const.tile([128, D_H], L1_DT)
        dma_w = nc.gpsimd if L1_DT != F32 else nc.sync
        dma_w.dma_start(We_sb[:], We_ext[:])

        Wm_sb = const.tile([128, 2, D_OUT], L2_DT)
        Wl_sb = const.tile([128, 2, D_OUT], L2_DT)
        dma_w2 = nc.gpsimd if L2_DT != F32 else nc.sync
        dma_w2.dma_start(Wm_sb[:], Wm_ext.rearrange("(c p) d -> p c d", p=128))
        dma_w2.dma_start(Wl_sb[:], Wl_ext.rearrange("(c p) d -> p c d", p=128))

        be_sb = const.tile([128, 2], F32)
        nc.sync.dma_start(be_sb[:], be_ext.rearrange("(c p) -> p c", p=128))

        ones_sb = const.tile([1, 128], F32)
        nc.vector.memset(ones_sb[:], 1.0)
        bm_rep = const.tile([1, S * D_OUT], F32)
        bl_rep = const.tile([1, S * D_OUT], F32)
        for s in range(S):
            nc.sync.dma_start(
                bm_rep[0:1, s * D_OUT : (s + 1) * D_OUT],
                bm_ext.rearrange("(o d) -> o d", o=1),
            )
            nc.sync.dma_start(
                bl_rep[0:1, s * D_OUT : (s + 1) * D_OUT],
                bl_ext.rearrange("(o d) -> o d", o=1),
            )

        for t in range(n_tiles):
            x_sb = xin.tile([128, S, D_IN], F32, tag="x")
            nc.sync.dma_start(x_sb[:], xv[t])
            eps_sb = epool.tile([128, S, D_OUT], F32, tag="eps")
            nc.sync.dma_start(eps_sb[:], ev[t])

            # transpose x -> xT [d_in, rows]
            xT_ps = psA.tile([128, ROWS_PER_TILE], F32, tag="xT")
            for s in range(S):
                nc.tensor.transpose(
                    xT_ps[:, s * 128 : (s + 1) * 128], x_sb[:, s, :], ident[:]
                )
            xT_sb = xTp.tile([128, ROWS_PER_TILE], L1_DT, tag="xTs")
            nc.vector.tensor_copy(xT_sb[:], xT_ps[:])

            # layer 1: hT[c] = W_emb[:, c].T @ xT  (c: two 128-wide d_h chunks)
            hT_ps0 = psB.tile([128, ROWS_PER_TILE], F32, tag="hT0")
            hT_ps1 = psB.tile([128, ROWS_PER_TILE], F32, tag="hT1")
            nc.tensor.matmul(
                hT_ps0[:], We_sb[:, 0:128], xT_sb[:], start=True, stop=True
            )
            nc.tensor.matmul(
                hT_ps1[:], We_sb[:, 128:256], xT_sb[:], start=True, stop=True
            )
            hT_sb0 = hTp.tile([128, ROWS_PER_TILE], L2_DT, tag="h0")
            hT_sb1 = hTp.tile([128, ROWS_PER_TILE], L2_DT, tag="h1")
            nc.scalar.activation(
                hT_sb0[:],
                hT_ps0[:],
                mybir.ActivationFunctionType.Relu,
                bias=be_sb[:, 0:1],
            )
            nc.scalar.activation(
                hT_sb1[:],
                hT_ps1[:],
                mybir.ActivationFunctionType.Relu,
                bias=be_sb[:, 1:2],
            )

            # layer 2: mean/logvar [rows, d_out] per 128-row subtile,
            # bias seeded across the whole 512-wide bank by a K=1 matmul
            mean_ps = psC.tile([128, S * D_OUT], F32, tag="mean")
            lv_ps = psC.tile([128, S * D_OUT], F32, tag="lv")
            nc.tensor.matmul(
                mean_ps[:], ones_sb[:], bm_rep[:],
                start=True, stop=False, skip_group_check=True,
            )
            nc.tensor.matmul(
                lv_ps[:], ones_sb[:], bl_rep[:],
                start=True, stop=False, skip_group_check=True,
            )
            for s in range(S):
                sl = slice(s * 128, (s + 1) * 128)
                so = slice(s * D_OUT, (s + 1) * D_OUT)
                nc.tensor.matmul(
                    mean_ps[:, so], hT_sb0[:, sl], Wm_sb[:, 0, :],
                    start=False, stop=False, skip_group_check=True,
                )
                nc.tensor.matmul(
                    mean_ps[:, so], hT_sb1[:, sl], Wm_sb[:, 1, :],
                    start=False, stop=(s == S - 1), skip_group_check=True,
                )
                nc.tensor.matmul(
                    lv_ps[:, so], hT_sb0[:, sl], Wl_sb[:, 0, :],
                    start=False, stop=False, skip_group_check=True,
                )
                nc.tensor.matmul(
                    lv_ps[:, so], hT_sb1[:, sl], Wl_sb[:, 1, :],
                    start=False, stop=(s == S - 1), skip_group_check=True,
                )

            # epilogue
            lv_sb = outs.tile([128, S * D_OUT], F32, tag="lvs")
            nc.scalar.activation(
                lv_sb[:], lv_ps[:], mybir.ActivationFunctionType.Copy
            )
            std_sb = outs.tile([128, S * D_OUT], F32, tag="std")
            nc.scalar.activation(
                std_sb[:], lv_ps[:], mybir.ActivationFunctionType.Exp, scale=0.5
            )
            mean_sb = outs.tile([128, S * D_OUT], F32, tag="means")
            nc.vector.tensor_copy(mean_sb[:], mean_ps[:])
            se_sb = outs.tile([128, S * D_OUT], F32, tag="se")
            nc.vector.tensor_mul(
                se_sb[:], std_sb[:], eps_sb[:].rearrange("p s d -> p (s d)")
            )
            z_sb = outs.tile([128, S * D_OUT], F32, tag="z")
            nc.vector.scalar_tensor_tensor(
                z_sb[:], mean_ps[:], 1.0, se_sb[:], AluOpType.mult, AluOpType.add
            )

            nc.sync.dma_start(zv[t], z_sb[:].rearrange("p (s d) -> p s d", s=S))
            nc.sync.dma_start(mv[t], mean_sb[:].rearrange("p (s d) -> p s d", s=S))
            nc.sync.dma_start(lvv[t], lv_sb[:].rearrange("p (s d) -> p s d", s=S))

    nc.finalize()
    return nc


_NC_CACHE = None


def _get_nc():
    global _NC_CACHE
    if _NC_CACHE is None:
        _NC_CACHE = build_bass()
    return _NC_CACHE


def _run(inputs, trace=False, **kw):
    nc = _get_nc()
    xs = np.ascontiguousarray(np.asarray(inputs["x"], dtype=np.float32))
    es = np.ascontiguousarray(np.asarray(inputs["eps"], dtype=np.float32))
    weights = {
        k: np.ascontiguousarray(np.asarray(inputs[k], dtype=np.float32))
        for k in ("W_emb", "b_emb", "W_mean", "b_mean", "W_logvar", "b_logvar")
    }
    in_maps = []
    for c in range(N_CORES):
        sl = slice(c * ROWS_PER_CORE, (c + 1) * ROWS_PER_CORE)
        in_maps.append({"x": xs[sl], "eps": es[sl], **weights})
    res = run_bass_kernel_spmd(nc, in_maps, list(range(N_CORES)), trace=trace, **kw)
    z = np.concatenate([res.results[c]["z"] for c in range(N_CORES)], axis=0)
    mean = np.concatenate([res.results[c]["mean"] for c in range(N_CORES)], axis=0)
    lv = np.concatenate([res.results[c]["logvar"] for c in range(N_CORES)], axis=0)
    return (z, mean, lv), res


def kernel(**inputs):
    out, _ = _run(inputs, trace=False)
    return out


if __name__ == "__main__":
    rng = np.random.default_rng(0)
    demo = {
        "x": rng.standard_normal((B, D_IN), dtype=np.float32),
        "eps": rng.standard_normal((B, D_OUT), dtype=np.float32),
        "W_emb": rng.standard_normal((D_IN, D_H), dtype=np.float32) * 0.088,
        "b_emb": rng.standard_normal((D_H,), dtype=np.float32) * 0.05,
        "W_mean": rng.standard_normal((D_H, D_OUT), dtype=np.float32) * 0.06,
        "b_mean": rng.standard_normal((D_OUT,), dtype=np.float32) * 0.03,
        "W_logvar": rng.standard_normal((D_H, D_OUT), dtype=np.float32) * 0.06,
        "b_logvar": rng.standard_normal((D_OUT,), dtype=np.float32) * 0.03,
    }
    z, m, l = kernel(**demo)
    print("shapes", z.shape, m.shape, l.shape)



# revision 4
# speedup vs baseline: 3.6122x; 3.6122x over previous
"""GaussianMLP sampling kernel for 8 trn2 NeuronCores (pure data parallel).

reference:
    h      = relu(x @ W_emb + b_emb)        x:[B,128] W_emb:[128,256]
    mean   = h @ W_mean + b_mean            W_mean:[256,128]
    logvar = h @ W_logvar + b_logvar        W_logvar:[256,128]
    z      = mean + exp(0.5*logvar) * eps
    returns (z, mean, logvar)

Fully transposed dataflow: the host pre-transposes x/eps per core to
[d, rows] bf16, the device computes everything in [feature, row] space
(features on partitions), and the host transposes the three bf16
outputs back.  Wins vs. the row-major formulation:
  - no PE transposes and no K=1 bias-seed matmuls: every bias is
    per-partition, folded into ACT/DVE epilogue ops for free
  - all matmuls are bf16 with the small weight as the stationary
    operand and 512-row moving tiles (N=512)
  - all five HBM streams are bf16 (halves traffic vs f32) and fully
    contiguous: 1 MiB DMAs with 8 KiB per-partition runs

Per 512-row compute tile:
  PE : hT0/hT1 = We_c.T @ xT      (2 MM, N=512, PSUM)
       mT/lT  += Wm_c.T/Wl_c.T @ hc  (4 MM accumulating, N=512)
  ACT: h0 = relu(hp0 + be0)  [PSUM->SBUF bf16]
       std = exp(0.5*lp + 0.5*bl)
  DVE: h1 = max(hp1 + be1, 0); mean = mp + bm; se = std*eps; z = mean+se
  GpS: logvar = lp + bl
DMA granules of 4096 rows (8 tiles) per stream, double buffered.
"""

import sys

sys.path.insert(0, "/opt/trn_rl_repo")

import numpy as np
import ml_dtypes

from contextlib import ExitStack

from concourse import bacc, mybir, tile
from concourse.alu_op_type import AluOpType
from concourse.bass_utils import run_bass_kernel_spmd

BF16_NP = ml_dtypes.bfloat16

N_CORES = 8
B = 524288
D_IN = 128
D_H = 256
D_OUT = 128
ROWS_PER_CORE = B // N_CORES  # 65536 (= columns per core in transposed space)
GCOLS = 4096  # DMA granule width (columns)
N_G = ROWS_PER_CORE // GCOLS  # 16
TCOLS = 512  # compute tile width (one PSUM bank)
N_T = GCOLS // TCOLS  # 8

F32 = mybir.dt.float32
BF16 = mybir.dt.bfloat16

Relu = mybir.ActivationFunctionType.Relu
Exp = mybir.ActivationFunctionType.Exp


def build_bass(cols_per_core=ROWS_PER_CORE):
    nc = bacc.Bacc("TRN2", target_bir_lowering=False, debug=False)
    n_g = cols_per_core // GCOLS

    xT = nc.declare_dram_parameter("xT", [D_IN, cols_per_core], BF16, isOutput=False)
    eT = nc.declare_dram_parameter("epsT", [D_OUT, cols_per_core], BF16, isOutput=False)
    We_ext = nc.declare_dram_parameter("W_emb", [D_IN, D_H], F32, isOutput=False)
    be_ext = nc.declare_dram_parameter("b_emb", [D_H], F32, isOutput=False)
    Wm_ext = nc.declare_dram_parameter("W_mean", [D_H, D_OUT], F32, isOutput=False)
    bm_ext = nc.declare_dram_parameter("b_mean", [D_OUT], F32, isOutput=False)
    Wl_ext = nc.declare_dram_parameter("W_logvar", [D_H, D_OUT], F32, isOutput=False)
    bl_ext = nc.declare_dram_parameter("b_logvar", [D_OUT], F32, isOutput=False)
    zT = nc.declare_dram_parameter("zT", [D_OUT, cols_per_core], BF16, isOutput=True)
    mT = nc.declare_dram_parameter("meanT", [D_OUT, cols_per_core], BF16, isOutput=True)
    lT = nc.declare_dram_parameter("lvT", [D_OUT, cols_per_core], BF16, isOutput=True)

    xv = xT.rearrange("d (g c) -> g d c", c=GCOLS)
    ev = eT.rearrange("d (g c) -> g d c", c=GCOLS)
    zv = zT.rearrange("d (g c) -> g d c", c=GCOLS)
    mv = mT.rearrange("d (g c) -> g d c", c=GCOLS)
    lv = lT.rearrange("d (g c) -> g d c", c=GCOLS)

    with tile.TileContext(nc) as tc, ExitStack() as ctx:
        const = ctx.enter_context(tc.tile_pool(name="const", bufs=1))
        xin = ctx.enter_context(tc.tile_pool(name="xin", bufs=2))
        ein = ctx.enter_context(tc.tile_pool(name="ein", bufs=2))
        outp = ctx.enter_context(tc.tile_pool(name="outp", bufs=2))
        hpool = ctx.enter_context(tc.tile_pool(name="hs", bufs=2))
        spool = ctx.enter_context(tc.tile_pool(name="sp", bufs=3))
        psH = ctx.enter_context(tc.tile_pool(name="psH", bufs=2, space="PSUM"))
        psO = ctx.enter_context(tc.tile_pool(name="psO", bufs=2, space="PSUM"))

        # --- weights / biases (loaded once, bf16 via SWDGE cast-DMA) ---
        We_sb = const.tile([128, D_H], BF16)
        nc.gpsimd.dma_start(We_sb[:], We_ext[:])
        Wm_sb = const.tile([128, 2, D_OUT], BF16)
        Wl_sb = const.tile([128, 2, D_OUT], BF16)
        nc.gpsimd.dma_start(Wm_sb[:], Wm_ext.rearrange("(c p) d -> p c d", p=128))
        nc.gpsimd.dma_start(Wl_sb[:], Wl_ext.rearrange("(c p) d -> p c d", p=128))

        be_sb = const.tile([128, 2], F32)
        nc.sync.dma_start(be_sb[:], be_ext.rearrange("(c p) -> p c", p=128))
        bm_sb = const.tile([128, 1], F32)
        nc.sync.dma_start(bm_sb[:], bm_ext.rearrange("(p o) -> p o", o=1))
        bl_sb = const.tile([128, 1], F32)
        nc.sync.dma_start(bl_sb[:], bl_ext.rearrange("(p o) -> p o", o=1))
        blh_sb = const.tile([128, 1], F32)
        nc.vector.tensor_scalar_mul(blh_sb[:], bl_sb[:], 0.5)

        x_tiles = [None] * n_g
        e_tiles = [None] * n_g

        def load_granule(g):
            x_tiles[g] = xin.tile([128, GCOLS], BF16, tag="x", name="xg")
            nc.sync.dma_start(x_tiles[g][:], xv[g])
            e_tiles[g] = ein.tile([128, GCOLS], BF16, tag="e", name="eg")
            nc.sync.dma_start(e_tiles[g][:], ev[g])

        load_granule(0)
        for g in range(n_g):
            if g + 1 < n_g:
                load_granule(g + 1)
            xg = x_tiles[g]
            eg = e_tiles[g]
            x_tiles[g] = e_tiles[g] = None
            zg = outp.tile([128, GCOLS], BF16, tag="z")
            mg = outp.tile([128, GCOLS], BF16, tag="m")
            lg = outp.tile([128, GCOLS], BF16, tag="l")
            for t in range(N_T):
                sl = slice(t * TCOLS, (t + 1) * TCOLS)
                hp0 = psH.tile([128, TCOLS], F32, tag="hp0")
                hp1 = psH.tile([128, TCOLS], F32, tag="hp1")
                nc.tensor.matmul(hp0[:], We_sb[:, 0:128], xg[:, sl], start=True, stop=True)
                nc.tensor.matmul(hp1[:], We_sb[:, 128:256], xg[:, sl], start=True, stop=True)

                h0 = hpool.tile([128, TCOLS], BF16, tag="h0")
                h1 = hpool.tile([128, TCOLS], BF16, tag="h1")
                # relu(h + be): chunk 0 on ACT, chunk 1 on DVE
                nc.scalar.activation(h0[:], hp0[:], Relu, bias=be_sb[:, 0:1])
                nc.vector.tensor_scalar(
                    h1[:], hp1[:], be_sb[:, 1:2], 0.0, AluOpType.add, AluOpType.max
                )

                mp = psO.tile([128, TCOLS], F32, tag="mp")
                lp = psO.tile([128, TCOLS], F32, tag="lp")
                nc.tensor.matmul(mp[:], Wm_sb[:, 0, :], h0[:], start=True, stop=False)
                nc.tensor.matmul(mp[:], Wm_sb[:, 1, :], h1[:], start=False, stop=True)
                nc.tensor.matmul(lp[:], Wl_sb[:, 0, :], h0[:], start=True, stop=False)
                nc.tensor.matmul(lp[:], Wl_sb[:, 1, :], h1[:], start=False, stop=True)

                # epilogue (GpSimd cannot read PSUM: it gets the SBUF-only ops)
                nc.vector.tensor_scalar_add(mg[:, sl], mp[:], bm_sb[:, 0:1])
                nc.vector.tensor_scalar_add(lg[:, sl], lp[:], bl_sb[:, 0:1])
                std = spool.tile([128, TCOLS], BF16, tag="std")
                nc.scalar.activation(std[:], lp[:], Exp, bias=blh_sb[:, 0:1], scale=0.5)
                se = spool.tile([128, TCOLS], BF16, tag="se")
                nc.gpsimd.tensor_mul(se[:], std[:], eg[:, sl])
                nc.gpsimd.tensor_add(zg[:, sl], mg[:, sl], se[:])

            nc.sync.dma_start(mv[g], mg[:])
            nc.sync.dma_start(lv[g], lg[:])
            nc.sync.dma_start(zv[g], zg[:])

    nc.finalize()
    return nc


_NC_CACHE = None


def _get_nc():
    global _NC_CACHE
    if _NC_CACHE is None:
        _NC_CACHE = build_bass()
    return _NC_CACHE


def _run(inputs, trace=False, **kw):
    nc = _get_nc()
    xs = np.asarray(inputs["x"], dtype=np.float32)
    es = np.asarray(inputs["eps"], dtype=np.float32)
    weights = {
        k: np.ascontiguousarray(np.asarray(inputs[k], dtype=np.float32))
        for k in ("W_emb", "b_emb", "W_mean", "b_mean", "W_logvar", "b_logvar")
    }
    in_maps = []
    for c in range(N_CORES):
        sl = slice(c * ROWS_PER_CORE, (c + 1) * ROWS_PER_CORE)
        in_maps.append(
            {
                "xT": xs[sl].T.astype(BF16_NP, order="C"),
                "epsT": es[sl].T.astype(BF16_NP, order="C"),
                **weights,
            }
        )
    res = run_bass_kernel_spmd(nc, in_maps, list(range(N_CORES)), trace=trace, **kw)
    z = np.empty((B, D_OUT), np.float32)
    mean = np.empty((B, D_OUT), np.float32)
    logvar = np.empty((B, D_OUT), np.float32)
    for c in range(N_CORES):
        sl = slice(c * ROWS_PER_CORE, (c + 1) * ROWS_PER_CORE)
        z[sl] = res.results[c]["zT"].T
        mean[sl] = res.results[c]["meanT"].T
        logvar[sl] = res.results[c]["lvT"].T
    return (z, mean, logvar), res


def kernel(**inputs):
    out, _ = _run(inputs, trace=False)
    return out


if __name__ == "__main__":
    rng = np.random.default_rng(0)
    demo = {
        "x": rng.standard_normal((B, D_IN), dtype=np.float32),
        "eps": rng.standard_normal((B, D_OUT), dtype=np.float32),
        "W_emb": rng.standard_normal((D_IN, D_H), dtype=np.float32) * 0.088,
        "b_emb": rng.standard_normal((D_H,), dtype=np.float32) * 0.05,
        "W_mean": rng.standard_normal((D_H, D_OUT), dtype=np.float32) * 0.06,
        "b_mean": rng.standard_normal((D_OUT,), dtype=np.float32) * 0.03,
        "W_logvar": rng.standard_normal((D_H, D_OUT), dtype=np.float32) * 0.06,
        "b_logvar": rng.standard_normal((D_OUT,), dtype=np.float32) * 0.03,
    }
    z, m, l = kernel(**demo)
    print("shapes", z.shape, m.shape, l.shape)


# revision 6
# speedup vs baseline: 4.3051x; 1.1918x over previous
"""GaussianMLP sampling kernel for 8 trn2 NeuronCores (pure data parallel).

reference:
    h      = relu(x @ W_emb + b_emb)        x:[B,128] W_emb:[128,256]
    mean   = h @ W_mean + b_mean            W_mean:[256,128]
    logvar = h @ W_logvar + b_logvar        W_logvar:[256,128]
    z      = mean + exp(0.5*logvar) * eps
    returns (z, mean, logvar)

Fully transposed dataflow: the host pre-transposes x/eps per core to
[d, rows] bf16, the device computes everything in [feature, row] space
(features on partitions), and the host transposes the three bf16
outputs back.  Wins vs. the row-major formulation:
  - no PE transposes and no K=1 bias-seed matmuls: every bias is
    per-partition, folded into ACT/DVE epilogue ops for free
  - all matmuls are bf16 with the small weight as the stationary
    operand and 512-row moving tiles (N=512)
  - all five HBM streams are bf16 (halves traffic vs f32) and fully
    contiguous: 1 MiB DMAs with 8 KiB per-partition runs

Per 512-row compute tile:
  PE : hT0/hT1 = We_c.T @ xT      (2 MM, N=512, PSUM)
       mT/lT  += Wm_c.T/Wl_c.T @ hc  (4 MM accumulating, N=512)
  ACT: h0 = relu(hp0 + be0)  [PSUM->SBUF bf16]
       std = exp(0.5*lp + 0.5*bl)
  DVE: h1 = max(hp1 + be1, 0); mean = mp + bm; se = std*eps; z = mean+se
  GpS: logvar = lp + bl
DMA granules of 4096 rows (8 tiles) per stream, double buffered.
"""

import sys

sys.path.insert(0, "/opt/trn_rl_repo")

import numpy as np
import ml_dtypes

from contextlib import ExitStack

from concourse import bacc, mybir, tile
from concourse.alu_op_type import AluOpType
from concourse.bass_utils import run_bass_kernel_spmd

BF16_NP = ml_dtypes.bfloat16

N_CORES = 8
B = 524288
D_IN = 128
D_H = 256
D_OUT = 128
ROWS_PER_CORE = B // N_CORES  # 65536 (= columns per core in transposed space)
GCOLS = 4096  # DMA granule width (columns)
N_G = ROWS_PER_CORE // GCOLS  # 16
TCOLS = 512  # compute tile width (one PSUM bank)
N_T = GCOLS // TCOLS  # 8

F32 = mybir.dt.float32
BF16 = mybir.dt.bfloat16

Relu = mybir.ActivationFunctionType.Relu
Exp = mybir.ActivationFunctionType.Exp
Identity = mybir.ActivationFunctionType.Identity


def build_bass(cols_per_core=ROWS_PER_CORE):
    nc = bacc.Bacc("TRN2", target_bir_lowering=False, debug=False)
    n_g = cols_per_core // GCOLS

    xT = nc.declare_dram_parameter("xT", [D_IN, cols_per_core], BF16, isOutput=False)
    eT = nc.declare_dram_parameter("epsT", [D_OUT, cols_per_core], BF16, isOutput=False)
    We_ext = nc.declare_dram_parameter("W_emb", [D_IN, D_H], F32, isOutput=False)
    be_ext = nc.declare_dram_parameter("b_emb", [D_H], F32, isOutput=False)
    Wm_ext = nc.declare_dram_parameter("W_mean", [D_H, D_OUT], F32, isOutput=False)
    bm_ext = nc.declare_dram_parameter("b_mean", [D_OUT], F32, isOutput=False)
    Wl_ext = nc.declare_dram_parameter("W_logvar", [D_H, D_OUT], F32, isOutput=False)
    bl_ext = nc.declare_dram_parameter("b_logvar", [D_OUT], F32, isOutput=False)
    zT = nc.declare_dram_parameter("zT", [D_OUT, cols_per_core], BF16, isOutput=True)
    mT = nc.declare_dram_parameter("meanT", [D_OUT, cols_per_core], BF16, isOutput=True)
    lT = nc.declare_dram_parameter("lvT", [D_OUT, cols_per_core], BF16, isOutput=True)

    xv = xT.rearrange("d (g c) -> g d c", c=GCOLS)
    ev = eT.rearrange("d (g c) -> g d c", c=GCOLS)
    zv = zT.rearrange("d (g c) -> g d c", c=GCOLS)
    mv = mT.rearrange("d (g c) -> g d c", c=GCOLS)
    lv = lT.rearrange("d (g c) -> g d c", c=GCOLS)

    with tile.TileContext(nc) as tc, ExitStack() as ctx:
        const = ctx.enter_context(tc.tile_pool(name="const", bufs=1))
        xin = ctx.enter_context(tc.tile_pool(name="xin", bufs=2))
        ein = ctx.enter_context(tc.tile_pool(name="ein", bufs=2))
        outp = ctx.enter_context(tc.tile_pool(name="outp", bufs=2))
        hpool = ctx.enter_context(tc.tile_pool(name="hs", bufs=2))
        spool = ctx.enter_context(tc.tile_pool(name="sp", bufs=3))
        psH = ctx.enter_context(tc.tile_pool(name="psH", bufs=2, space="PSUM"))
        psO = ctx.enter_context(tc.tile_pool(name="psO", bufs=2, space="PSUM"))

        # --- weights / biases (loaded once, bf16 via SWDGE cast-DMA) ---
        We_sb = const.tile([128, D_H], BF16)
        nc.gpsimd.dma_start(We_sb[:], We_ext[:])
        Wm_sb = const.tile([128, 2, D_OUT], BF16)
        Wl_sb = const.tile([128, 2, D_OUT], BF16)
        nc.gpsimd.dma_start(Wm_sb[:], Wm_ext.rearrange("(c p) d -> p c d", p=128))
        nc.gpsimd.dma_start(Wl_sb[:], Wl_ext.rearrange("(c p) d -> p c d", p=128))

        be_sb = const.tile([128, 2], F32)
        nc.sync.dma_start(be_sb[:], be_ext.rearrange("(c p) -> p c", p=128))
        bm_sb = const.tile([128, 1], F32)
        nc.sync.dma_start(bm_sb[:], bm_ext.rearrange("(p o) -> p o", o=1))
        bl_sb = const.tile([128, 1], F32)
        nc.sync.dma_start(bl_sb[:], bl_ext.rearrange("(p o) -> p o", o=1))
        blh_sb = const.tile([128, 1], F32)
        nc.vector.tensor_scalar_mul(blh_sb[:], bl_sb[:], 0.5)

        x_tiles = [None] * n_g
        e_tiles = [None] * n_g

        def load_granule(g):
            x_tiles[g] = xin.tile([128, GCOLS], BF16, tag="x", name="xg")
            nc.sync.dma_start(x_tiles[g][:], xv[g])
            e_tiles[g] = ein.tile([128, GCOLS], BF16, tag="e", name="eg")
            nc.sync.dma_start(e_tiles[g][:], ev[g])

        load_granule(0)
        for g in range(n_g):
            if g + 1 < n_g:
                load_granule(g + 1)
            xg = x_tiles[g]
            eg = e_tiles[g]
            x_tiles[g] = e_tiles[g] = None
            zg = outp.tile([128, GCOLS], BF16, tag="z")
            mg = outp.tile([128, GCOLS], BF16, tag="m")
            lg = outp.tile([128, GCOLS], BF16, tag="l")
            for t in range(N_T):
                sl = slice(t * TCOLS, (t + 1) * TCOLS)
                hp0 = psH.tile([128, TCOLS], F32, tag="hp0")
                hp1 = psH.tile([128, TCOLS], F32, tag="hp1")
                nc.tensor.matmul(hp0[:], We_sb[:, 0:128], xg[:, sl], start=True, stop=True)
                nc.tensor.matmul(hp1[:], We_sb[:, 128:256], xg[:, sl], start=True, stop=True)

                h0 = hpool.tile([128, TCOLS], BF16, tag="h0")
                h1 = hpool.tile([128, TCOLS], BF16, tag="h1")
                # relu(h + be): chunk 0 on ACT, chunk 1 on DVE
                nc.scalar.activation(h0[:], hp0[:], Relu, bias=be_sb[:, 0:1])
                nc.vector.tensor_scalar(
                    h1[:], hp1[:], be_sb[:, 1:2], 0.0, AluOpType.add, AluOpType.max
                )

                mp = psO.tile([128, TCOLS], F32, tag="mp")
                lp = psO.tile([128, TCOLS], F32, tag="lp")
                nc.tensor.matmul(mp[:], Wm_sb[:, 0, :], h0[:], start=True, stop=False)
                nc.tensor.matmul(mp[:], Wm_sb[:, 1, :], h1[:], start=False, stop=True)
                nc.tensor.matmul(lp[:], Wl_sb[:, 0, :], h0[:], start=True, stop=False)
                nc.tensor.matmul(lp[:], Wl_sb[:, 1, :], h1[:], start=False, stop=True)

                # epilogue (GpSimd cannot read PSUM: it gets the SBUF-only z)
                # mean granule holds mean WITHOUT b_mean; host adds b_mean to
                # both mean and z during reassembly.
                nc.vector.tensor_copy(mg[:, sl], mp[:])
                nc.scalar.activation(
                    lg[:, sl], lp[:], Identity, bias=bl_sb[:, 0:1]
                )
                std = spool.tile([128, TCOLS], BF16, tag="std")
                nc.scalar.activation(std[:], lp[:], Exp, bias=blh_sb[:, 0:1], scale=0.5)
                se = spool.tile([128, TCOLS], BF16, tag="se")
                nc.vector.tensor_mul(se[:], std[:], eg[:, sl])
                nc.gpsimd.tensor_add(zg[:, sl], mg[:, sl], se[:])

            nc.sync.dma_start(mv[g], mg[:])
            nc.sync.dma_start(lv[g], lg[:])
            nc.sync.dma_start(zv[g], zg[:])

    nc.finalize()
    return nc


_NC_CACHE = None


def _get_nc():
    global _NC_CACHE
    if _NC_CACHE is None:
        _NC_CACHE = build_bass()
    return _NC_CACHE


def _run(inputs, trace=False, **kw):
    nc = _get_nc()
    xs = np.asarray(inputs["x"], dtype=np.float32)
    es = np.asarray(inputs["eps"], dtype=np.float32)
    weights = {
        k: np.ascontiguousarray(np.asarray(inputs[k], dtype=np.float32))
        for k in ("W_emb", "b_emb", "W_mean", "b_mean", "W_logvar", "b_logvar")
    }
    in_maps = []
    for c in range(N_CORES):
        sl = slice(c * ROWS_PER_CORE, (c + 1) * ROWS_PER_CORE)
        in_maps.append(
            {
                "xT": xs[sl].T.astype(BF16_NP, order="C"),
                "epsT": es[sl].T.astype(BF16_NP, order="C"),
                **weights,
            }
        )
    res = run_bass_kernel_spmd(nc, in_maps, list(range(N_CORES)), trace=trace, **kw)
    z = np.empty((B, D_OUT), np.float32)
    mean = np.empty((B, D_OUT), np.float32)
    logvar = np.empty((B, D_OUT), np.float32)
    bm = weights["b_mean"]
    for c in range(N_CORES):
        sl = slice(c * ROWS_PER_CORE, (c + 1) * ROWS_PER_CORE)
        np.add(res.results[c]["zT"].T, bm, out=z[sl])
        np.add(res.results[c]["meanT"].T, bm, out=mean[sl])
        logvar[sl] = res.results[c]["lvT"].T
    return (z, mean, logvar), res


def kernel(**inputs):
    out, _ = _run(inputs, trace=False)
    return out


if __name__ == "__main__":
    rng = np.random.default_rng(0)
    demo = {
        "x": rng.standard_normal((B, D_IN), dtype=np.float32),
        "eps": rng.standard_normal((B, D_OUT), dtype=np.float32),
        "W_emb": rng.standard_normal((D_IN, D_H), dtype=np.float32) * 0.088,
        "b_emb": rng.standard_normal((D_H,), dtype=np.float32) * 0.05,
        "W_mean": rng.standard_normal((D_H, D_OUT), dtype=np.float32) * 0.06,
        "b_mean": rng.standard_normal((D_OUT,), dtype=np.float32) * 0.03,
        "W_logvar": rng.standard_normal((D_H, D_OUT), dtype=np.float32) * 0.06,
        "b_logvar": rng.standard_normal((D_OUT,), dtype=np.float32) * 0.03,
    }
    z, m, l = kernel(**demo)
    print("shapes", z.shape, m.shape, l.shape)
